# revision 1
# baseline (speedup 1.0000x reference)
"""Trainium2 Bass kernel for nn_BoundaryLoss: mean(|softmax(pred) * SDF(onehot(target))|).

Strategy (8 NeuronCores, SPMD):
  - One (b, c) pair per core (B=2 x C=4 = 8 pairs). Each core computes the exact
    3D squared Euclidean distance transform of the class-c seed mask (pos) and its
    complement (neg) for its batch element, via separable truncated-shift min-plus
    passes (shift radius S derived from the input on the host; truncation at
    S >= max true distance is exact). |sdf| = sqrt(g_pos + g_neg) since exactly one
    of the two is zero at every voxel. The core then multiplies by softmax(pred)[c]
    and reduces to 48 per-partition partial sums.
  - Host shards inputs, sums the 8x48 partials, applies the has_pos gate and the
    1/(B*C*D*H*W) mean factor.

Layout per core: SBUF tiles [NP, 2304] with partition rows
  [0,S): INF border | [S, S+48): pos volume (row S+d) | [S+48, 2S+48): INF gap |
  [2S+48, 2S+96): neg volume | [2S+96, 3S+96): INF border
free dim = (h, w) flattened. W/H passes shift along free dims; the D pass uses
partition-offset SBUF->SBUF DMA copies (compute ops never straddle partitions).
EDT arithmetic is int16 (exact: all squared distances are integers <= 6627; INF
is 30000 and never overflows: 30000 + 3*47^2 < 32767).
"""

import os
import sys

import numpy as np

B, C, DD, HH, WW = 2, 4, 48, 48, 48
PLANE = HH * WW  # free size 2304
NVOX = DD * PLANE
INF16 = 30000.0
S_MAX = 16  # gap/tail rows bound the shift radius
N_CORES = 8

_nc_cache = {}
LAST_RESULTS = None  # test harness introspection


def _ensure_paths():
    for p in ("/opt/trn_rl_repo",):
        if os.path.isdir(p) and p not in sys.path:
            sys.path.insert(0, p)


def _edt_sq_trunc_np(f0, S):
    """Truncated-shift separable squared EDT (numpy, int32). Mirrors the device
    algorithm; used for the shift-bound certification and the fallback path."""
    f = f0.astype(np.int32)
    for ax in (2, 1, 0):
        g = f.copy()
        for s in range(1, S + 1):
            s2 = s * s
            sl_out = [slice(None)] * 3
            sl_in = [slice(None)] * 3
            sl_out[ax] = slice(s, None)
            sl_in[ax] = slice(None, -s)
            np.minimum(g[tuple(sl_out)], f[tuple(sl_in)] + s2, out=g[tuple(sl_out)])
            sl_out[ax] = slice(None, -s)
            sl_in[ax] = slice(s, None)
            np.minimum(g[tuple(sl_out)], f[tuple(sl_in)] + s2, out=g[tuple(sl_out)])
        f = g
    return f


def _certified_shift_bound(masks):
    """Smallest S such that the S-truncated separable EDT is provably exact for
    every seed mask in `masks`: if the truncated result's max distance is <= S,
    truncation never cut off a winning chain (truncation only overestimates, so
    max_true <= max_trunc <= S certifies S >= max per-axis seed offset)."""
    for S in range(1, S_MAX + 1):
        worst = 0
        for m in masks:
            f0 = np.where(m, 0, 30000).astype(np.int16)
            g = _edt_sq_trunc_np(f0, S)
            worst = max(worst, int(np.ceil(np.sqrt(float(g.max())))))
        if worst <= S:
            return S
    return S_MAX + 1  # triggers the fallback path


def _reference_fallback(pred, target):
    """Exact numpy replica of the reference for pathological inputs the device
    path does not cover (wrong shapes, empty masks, S > S_MAX)."""
    INF = 1e9
    pred = np.asarray(pred, np.float32)
    target = np.asarray(target)
    b_, c_ = pred.shape[0], pred.shape[1]
    n = np.arange(pred.shape[-1])

    def minplus(f):
        d2 = ((n[:, None] - n[None, :]) ** 2).astype(np.float32)
        return (f[..., None, :] + d2).min(axis=-1)

    def edt(src):
        f = np.where(src, 0.0, INF).astype(np.float32)
        for ax in (-3, -2, -1):
            f = np.moveaxis(minplus(np.moveaxis(f, ax, -1)), -1, ax)
        return np.sqrt(f)

    e = np.exp(pred - pred.max(axis=1, keepdims=True))
    sm = e / e.sum(axis=1, keepdims=True)
    total = 0.0
    for b in range(b_):
        for c in range(c_):
            pos = target[b] == c
            if not pos.any():
                continue
            sdf = edt(pos) - edt(~pos)
            total += float(np.abs(sm[b, c] * sdf).sum(dtype=np.float64))
    return np.float32(total / pred.size)


def _build_nc(S):
    """Build + compile the SPMD Bass program for shift radius S.

    Row layout (128 partitions; compute partition ranges must start naturally
    aligned: count<=32 -> 32-aligned start, <=64 -> 64-aligned, >64 -> start 0):
      [0,48) pos volume | [48,64) INF gap | [64,112) neg volume | [112,128) INF
    """
    _ensure_paths()
    import concourse.tile as tile
    from concourse import bacc, mybir

    i16 = mybir.dt.int16
    f32 = mybir.dt.float32
    ALU = mybir.AluOpType
    ACT = mybir.ActivationFunctionType

    NP = 128
    RB = 64            # neg block start row
    RV = 112           # end of valid rows (compute range [0, RV))

    nc = bacc.Bacc("TRN2", target_bir_lowering=False, debug=False)

    tgt_d = nc.dram_tensor("tgt", [NP, PLANE], i16, kind="ExternalInput")
    cv_d = nc.dram_tensor("cvec", [NP, 1], f32, kind="ExternalInput")
    pred_d = nc.dram_tensor("pred4", [C, DD, PLANE], f32, kind="ExternalInput")
    pm_d = nc.dram_tensor("pairmat", [NP, 48], f32, kind="ExternalInput")
    out_d = nc.dram_tensor("out", [48, 1], f32, kind="ExternalOutput")

    with tile.TileContext(nc) as tc:
        with (
            tc.tile_pool(name="main", bufs=1) as pool,
            tc.tile_pool(name="fsp", bufs=4) as fsp,
            tc.tile_pool(name="psum", bufs=1, space="PSUM") as psp,
        ):
            Tt = pool.tile([NP, PLANE], i16, tag="T")
            nc.sync.dma_start(Tt[:], tgt_d[:])
            CV = pool.tile([NP, 1], f32, tag="cv")
            nc.sync.dma_start(CV[:], cv_d[:])
            PM = pool.tile([NP, 48], f32, tag="pm")
            nc.sync.dma_start(PM[:], pm_d[:])
            PR = pool.tile([48, C * PLANE], f32, tag="pr")
            nc.sync.dma_start(PR[:], pred_d.rearrange("c p n -> p c n"))

            A = pool.tile([NP, PLANE], i16, tag="A")
            Bt = pool.tile([NP, PLANE], i16, tag="B")

            # onehot init: pos rows f = (t != c)*INF, neg rows f = (t == c)*INF.
            # Host sentinel rows make the gap come out INF; tail memset to INF.
            nc.gpsimd.memset(A[96:NP, :], INF16)
            nc.vector.tensor_scalar(
                out=A[0:RB, :], in0=Tt[0:RB, :], scalar1=CV[0:RB, :],
                scalar2=INF16, op0=ALU.not_equal, op1=ALU.mult,
            )
            nc.vector.tensor_scalar(
                out=A[RB:RV, :], in0=Tt[RB:RV, :], scalar1=CV[RB:RV, :],
                scalar2=INF16, op0=ALU.is_equal, op1=ALU.mult,
            )

            def freepass(src, dst, axis_w):
                """min-plus pass along w (axis_w=True) or h (False), src -> dst."""
                s3 = src[:].rearrange("p (h w) -> p h w", w=WW)
                d3 = dst[:].rearrange("p (h w) -> p h w", w=WW)
                nc.vector.tensor_copy(dst[0:RV, :], src[0:RV, :])
                for s in range(1, S + 1):
                    s2 = float(s * s)
                    if axis_w:
                        pairs = [
                            (d3[0:RV, :, s:], s3[0:RV, :, : WW - s]),
                            (d3[0:RV, :, : WW - s], s3[0:RV, :, s:]),
                        ]
                    else:
                        pairs = [
                            (d3[0:RV, s:, :], s3[0:RV, : HH - s, :]),
                            (d3[0:RV, : HH - s, :], s3[0:RV, s:, :]),
                        ]
                    for dap, sap in pairs:
                        nc.vector.scalar_tensor_tensor(
                            out=dap, in0=sap, scalar=s2, in1=dap,
                            op0=ALU.add, op1=ALU.min,
                        )

            freepass(A, Bt, axis_w=True)   # pass along W
            freepass(Bt, A, axis_w=False)  # pass along H

            # pass along D: partition-offset DMA copies + aligned STT updates.
            # A's gap/tail rows are INF so shifted reads never leak across blocks.
            nc.vector.tensor_copy(Bt[0:RV, :], A[0:RV, :])
            for s in range(1, S + 1):
                s2 = float(s * s)
                for sign in (1, -1):
                    fs = fsp.tile([NP, PLANE], i16, tag="fs")
                    if sign > 0:
                        nc.gpsimd.memset(fs[0:32, :], INF16)
                        nc.sync.dma_start(fs[s:NP, :], A[0 : NP - s, :])
                    else:
                        nc.gpsimd.memset(fs[96:NP, :], INF16)
                        nc.sync.dma_start(fs[0 : NP - s, :], A[s:NP, :])
                    nc.vector.scalar_tensor_tensor(
                        out=Bt[0:RV, :], in0=fs[0:RV, :], scalar=s2,
                        in1=Bt[0:RV, :], op0=ALU.add, op1=ALU.min,
                    )

            # |sdf| = sqrt(g_pos + g_neg): sqrt rows, then pair-sum via PE matmul
            SQ = pool.tile([NP, PLANE], f32, tag="SQ")
            nc.gpsimd.memset(SQ[96:NP, :], 0.0)
            nc.scalar.activation(SQ[0:RV, :], Bt[0:RV, :], ACT.Sqrt)
            PS = psp.tile([48, PLANE], f32, tag="ps")
            n0 = 0
            while n0 < PLANE:
                nn = min(512, PLANE - n0)
                nc.tensor.matmul(
                    PS[:, n0 : n0 + nn], PM[:], SQ[:, n0 : n0 + nn],
                    start=True, stop=True,
                )
                n0 += nn

            # softmax weight for class c (host permuted class c to slot 0)
            nc.scalar.activation(PR[:], PR[:], ACT.Exp)
            DN = pool.tile([48, PLANE], f32, tag="dn")
            nc.vector.tensor_tensor(DN[:], PR[:, 0:PLANE], PR[:, PLANE : 2 * PLANE], ALU.add)
            nc.vector.tensor_tensor(DN[:], DN[:], PR[:, 2 * PLANE : 3 * PLANE], ALU.add)
            nc.vector.tensor_tensor(DN[:], DN[:], PR[:, 3 * PLANE : 4 * PLANE], ALU.add)
            RC = pool.tile([48, PLANE], f32, tag="rc")
            nc.vector.reciprocal(RC[:], DN[:])
            nc.vector.tensor_tensor(DN[:], PR[:, 0:PLANE], RC[:], ALU.mult)

            # partial[d] = sum_(h,w) |sdf| * w_c
            AC = pool.tile([48, 1], f32, tag="ac")
            nc.vector.tensor_tensor(SQ[0:48, :], PS[:], DN[:], ALU.mult)
            nc.vector.reduce_sum(AC[:], SQ[0:48, :], axis=mybir.AxisListType.X)
            nc.sync.dma_start(out_d[:], AC[:])

    nc.compile()
    return nc


def kernel(pred, target):
    pred = np.ascontiguousarray(np.asarray(pred), dtype=np.float32)
    target = np.asarray(target)

    if pred.shape != (B, C, DD, HH, WW) or target.shape != (B, DD, HH, WW):
        return _reference_fallback(pred, target)

    tgt = target.astype(np.int64)
    masks = []
    has_pos = {}
    for b in range(B):
        for c in range(C):
            m = tgt[b] == c
            has_pos[(b, c)] = bool(m.any())
            if has_pos[(b, c)]:
                masks.append(m)
                mn = ~m
                if mn.any():
                    masks.append(mn)
                else:
                    return _reference_fallback(pred, target)  # class fills volume

    S = _certified_shift_bound(masks)
    if S > S_MAX:
        return _reference_fallback(pred, target)

    _ensure_paths()
    from concourse.bass_utils import run_bass_kernel_spmd

    if S not in _nc_cache:
        _nc_cache[S] = _build_nc(S)
    nc = _nc_cache[S]

    NP, RB = 128, 64

    pairmat = np.zeros((NP, 48), np.float32)
    pairmat[np.arange(48), np.arange(48)] = 1.0
    pairmat[RB + np.arange(48), np.arange(48)] = 1.0

    in_maps = []
    for k in range(N_CORES):
        b, c = divmod(k, C)
        t16 = tgt[b].reshape(DD, PLANE).astype(np.int16)
        T = np.empty((NP, PLANE), np.int16)
        T[0:48] = t16
        T[48:RB] = 5        # gap rows: != c -> INF
        T[RB : RB + 48] = t16
        T[RB + 48 :] = c    # unused tail rows
        cvec = np.full((NP, 1), c, np.float32)
        perm = [c] + [j for j in range(C) if j != c]
        pred4 = np.ascontiguousarray(pred[b][perm].reshape(C, DD, PLANE))
        in_maps.append({"tgt": T, "cvec": cvec, "pred4": pred4, "pairmat": pairmat})

    trace = bool(os.environ.get("BOUNDARY_KERNEL_TRACE"))
    if trace:
        import importlib.util

        if importlib.util.find_spec("antenv.axon_hooks") is None:
            trace = False  # NTFF hook unavailable in this axon build
    res = run_bass_kernel_spmd(nc, in_maps, list(range(N_CORES)), trace=trace)
    global LAST_RESULTS
    LAST_RESULTS = res

    total = 0.0
    for k in range(N_CORES):
        b, c = divmod(k, C)
        if has_pos[(b, c)]:
            total += float(res.results[k]["out"].astype(np.float64).sum())
    return np.float32(total / (B * C * NVOX))


if __name__ == "__main__":
    import reference

    inputs = reference.setup_inputs()
    out = kernel(**{k: np.asarray(v) for k, v in inputs.items()})
    print("kernel out:", out)



# revision 4
# speedup vs baseline: 2.7692x; 2.7692x over previous
"""Trainium2 Bass kernel for nn_BoundaryLoss: mean(|softmax(pred) * SDF(onehot(target))|).

Strategy (8 NeuronCores, SPMD, one (b, c) pair per core):
  Exact 3D squared EDT of the class mask and its complement via separable
  truncated-shift min-plus passes (radius S certified exact on host against an
  untruncated reference EDT). Both volumes share one [128, 2304] bf16 tile,
  gapless: rows [0,48) pos (partition=d), rows [48,96) neg, free=(h,w); the
  host bakes mask polarity in, so the device sees a single 0/1 seed tensor.

  Engine plan (v2 — replaces the DMA-shift/DVE-heavy v1):
   - H/W passes on DVE as tensor_scalar (src+s^2, 4x mode) + tensor_tensor min
     (2x_1p); scalar_tensor_tensor only for the odd W-shifts that break 4B
     alignment anyway (STT has no fast uop, both run 1x).
   - D pass (partition axis) on the PE: per shift a [97x128] bf16 matrix whose
     columns pick row p-s and add s^2 (or 30000=INF) via a constant-1 row at
     partition 96; min-accumulated on DVE from PSUM (partly via a Scalar
     engine bf16 relay).
   - softmax weight = exp(p_c - ln(denom)); denom = PE matmul with a
     duplicating selector matrix (sums the 4 class planes into both row
     blocks); Ln/Exp/Sqrt on the Scalar engine. No DVE reciprocal.
   - per-voxel |sdf|*w_c tensor DMA'd out (bf16); final reduce on host.
"""

import os
import sys

import numpy as np

B, C, DD, HH, WW = 2, 4, 48, 48, 48
PLANE = HH * WW  # 2304
NVOX = DD * PLANE
NP = 128
RB = 48          # neg block start row (gapless: pos [0,48), neg [48,96))
RV = 96          # end of valid rows
KROW = 96        # constant-1 row for the shift matmuls (contract dim 97)
INF = 30000.0
S_MAX = 6        # fall back to numpy beyond this
N_CORES = 8
HALF = PLANE // 2  # 1152

_nc_cache = {}
LAST_RESULTS = None  # test harness introspection


def _ensure_paths():
    for p in ("/opt/trn_rl_repo",):
        if os.path.isdir(p) and p not in sys.path:
            sys.path.insert(0, p)


def _edt_sq_trunc_np(f0, S):
    """Truncated-shift separable squared EDT (numpy, int32)."""
    f = f0.astype(np.int32)
    for ax in (2, 1, 0):
        g = f.copy()
        for s in range(1, S + 1):
            s2 = s * s
            sl_out = [slice(None)] * 3
            sl_in = [slice(None)] * 3
            sl_out[ax] = slice(s, None)
            sl_in[ax] = slice(None, -s)
            np.minimum(g[tuple(sl_out)], f[tuple(sl_in)] + s2, out=g[tuple(sl_out)])
            sl_out[ax] = slice(None, -s)
            sl_in[ax] = slice(s, None)
            np.minimum(g[tuple(sl_out)], f[tuple(sl_in)] + s2, out=g[tuple(sl_out)])
        f = g
    return f


def _certified_shift_bound(masks):
    """Smallest S whose S-truncated separable EDT equals an exact reference on
    every mask. The reference uses S_ref from the conservative bound (max
    truncated distance <= S_ref implies exactness); smaller S are accepted iff
    they reproduce the reference exactly (a per-input certificate)."""
    refs = []
    S_ref = None
    for S in range(1, 17):
        worst = 0
        refs = []
        for m in masks:
            f0 = np.where(m, 0, 30000).astype(np.int16)
            g = _edt_sq_trunc_np(f0, S)
            refs.append(g)
            worst = max(worst, int(np.ceil(np.sqrt(float(g.max())))))
        if worst <= S:
            S_ref = S
            break
    if S_ref is None:
        return 17
    for S in range(1, S_ref):
        ok = True
        for m, ref in zip(masks, refs):
            f0 = np.where(m, 0, 30000).astype(np.int16)
            if not np.array_equal(_edt_sq_trunc_np(f0, S), ref):
                ok = False
                break
        if ok:
            return S
    return S_ref


def _reference_fallback(pred, target):
    """Exact numpy replica of the reference for pathological inputs."""
    BIG = 1e9
    pred = np.asarray(pred, np.float32)
    target = np.asarray(target)
    b_, c_ = pred.shape[0], pred.shape[1]
    n = np.arange(pred.shape[-1])

    def minplus(f):
        d2 = ((n[:, None] - n[None, :]) ** 2).astype(np.float32)
        return (f[..., None, :] + d2).min(axis=-1)

    def edt(src):
        f = np.where(src, 0.0, BIG).astype(np.float32)
        for ax in (-3, -2, -1):
            f = np.moveaxis(minplus(np.moveaxis(f, ax, -1)), -1, ax)
        return np.sqrt(f)

    e = np.exp(pred - pred.max(axis=1, keepdims=True))
    sm = e / e.sum(axis=1, keepdims=True)
    total = 0.0
    for b in range(b_):
        for c in range(c_):
            pos = target[b] == c
            if not pos.any():
                continue
            sdf = edt(pos) - edt(~pos)
            total += float(np.abs(sm[b, c] * sdf).sum(dtype=np.float64))
    return np.float32(total / pred.size)


def _shift_list(S):
    out = []
    for s in range(1, S + 1):
        out.append(s)
        out.append(-s)
    return out


def _build_nc(S):
    """Build + compile the SPMD Bass program for shift radius S."""
    _ensure_paths()
    import concourse.tile as tile
    from concourse import bacc, mybir

    i16 = mybir.dt.int16
    bf16 = mybir.dt.bfloat16
    f32 = mybir.dt.float32
    ALU = mybir.AluOpType
    ACT = mybir.ActivationFunctionType

    shifts = _shift_list(S)
    NSH = len(shifts)
    RELAY = set(range(min(2, NSH)))  # shift idxs relayed PSUM->SBUF on Scalar

    nc = bacc.Bacc("TRN2", target_bir_lowering=False, debug=False)

    tgt_d = nc.dram_tensor("tgt", [NP, PLANE], i16, kind="ExternalInput")
    pr_d = nc.dram_tensor("prden", [NP, 2 * PLANE], bf16, kind="ExternalInput")
    pr2_d = nc.dram_tensor("prnum", [NP, PLANE], bf16, kind="ExternalInput")
    wm_d = nc.dram_tensor("wmats", [NP, 128 * (NSH + 1)], bf16, kind="ExternalInput")
    out_d = nc.dram_tensor("t2", [RV, PLANE], bf16, kind="ExternalOutput")

    with tile.TileContext(nc) as tc:
        with (
            tc.tile_pool(name="main", bufs=1) as pool,
            tc.tile_pool(name="psum", bufs=2, space="PSUM") as psp,
        ):
            Tt = pool.tile([NP, PLANE], i16, tag="T")
            nc.sync.dma_start(Tt[:], tgt_d[:])
            WM = pool.tile([NP, 128 * (NSH + 1)], bf16, tag="wm")
            nc.sync.dma_start(WM[:], wm_d[:])
            PR = pool.tile([NP, 2 * PLANE], bf16, tag="pr")
            nc.sync.dma_start(PR[:], pr_d[:])
            PR2 = pool.tile([NP, PLANE], bf16, tag="pr2")
            nc.sync.dma_start(PR2[:], pr2_d[:])

            A = pool.tile([NP, PLANE], bf16, tag="A")
            Bh = pool.tile([NP, PLANE], bf16, tag="Bh")
            Cw = pool.tile([NP, PLANE], bf16, tag="Cw")
            Dt = pool.tile([NP, PLANE], bf16, tag="Dt")

            # seed init: host bakes the onehot polarity; f = mask * INF.
            nc.vector.tensor_scalar(
                out=A[0:RV, :], in0=Tt[0:RV, :], scalar1=INF,
                scalar2=None, op0=ALU.mult,
            )

            # ---- H pass (free dim, stride-48 shifts stay 4B aligned) ----
            b3 = Bh[:].rearrange("p (h w) -> p h w", w=WW)
            nc.vector.tensor_copy(Bh[0:RV, :], A[0:RV, :])
            AS = pool.tile([NP, PLANE], bf16, tag="AS")
            as3 = AS[:].rearrange("p (h w) -> p h w", w=WW)
            for s in range(1, S + 1):
                nc.vector.tensor_scalar(
                    out=AS[0:RV, :], in0=A[0:RV, :], scalar1=float(s * s),
                    scalar2=None, op0=ALU.add,
                )
                nc.vector.tensor_tensor(
                    b3[0:RV, s:, :], as3[0:RV, : HH - s, :], b3[0:RV, s:, :], ALU.min
                )
                nc.vector.tensor_tensor(
                    b3[0:RV, : HH - s, :], as3[0:RV, s:, :], b3[0:RV, : HH - s, :],
                    ALU.min,
                )

            # ---- W pass (innermost dim; odd shifts lose 2x alignment) ----
            c3 = Cw[:].rearrange("p (h w) -> p h w", w=WW)
            bs3 = AS[:].rearrange("p (h w) -> p h w", w=WW)  # AS reused for Bh+s^2
            nc.vector.tensor_copy(Cw[0:RV, :], Bh[0:RV, :])
            for s in range(1, S + 1):
                s2 = float(s * s)
                if s % 2 == 0:
                    nc.vector.tensor_scalar(
                        out=AS[0:RV, :], in0=Bh[0:RV, :], scalar1=s2,
                        scalar2=None, op0=ALU.add,
                    )
                    nc.vector.tensor_tensor(
                        c3[0:RV, :, s:], bs3[0:RV, :, : WW - s], c3[0:RV, :, s:],
                        ALU.min,
                    )
                    nc.vector.tensor_tensor(
                        c3[0:RV, :, : WW - s], bs3[0:RV, :, s:],
                        c3[0:RV, :, : WW - s], ALU.min,
                    )
                else:
                    bh3 = Bh[:].rearrange("p (h w) -> p h w", w=WW)
                    nc.vector.scalar_tensor_tensor(
                        out=c3[0:RV, :, s:], in0=bh3[0:RV, :, : WW - s], scalar=s2,
                        in1=c3[0:RV, :, s:], op0=ALU.add, op1=ALU.min,
                    )
                    nc.vector.scalar_tensor_tensor(
                        out=c3[0:RV, :, : WW - s], in0=bh3[0:RV, :, s:], scalar=s2,
                        in1=c3[0:RV, :, : WW - s], op0=ALU.add, op1=ALU.min,
                    )

            # constant-1 rows feeding the +s^2 / INF terms of the shift matmuls
            nc.gpsimd.memset(Cw[KROW:NP, :], 1.0)

            # ---- softmax denom on PE + Ln on Scalar (overlaps EDT) ----
            E = pool.tile([NP, 2 * PLANE], bf16, tag="E")
            nc.scalar.activation(E[:], PR[:], ACT.Exp)
            L = pool.tile([NP, PLANE], bf16, tag="L")
            VV = WM[:, NSH * 128 : (NSH + 1) * 128]
            for h0 in range(0, PLANE, HALF):
                DP = psp.tile([NP, HALF], f32, tag="ps")
                for n0 in range(0, HALF, 512):
                    nn = min(512, HALF - n0)
                    nc.tensor.matmul(
                        DP[:, n0 : n0 + nn], VV, E[:, h0 + n0 : h0 + n0 + nn],
                        start=True, stop=False,
                    )
                    nc.tensor.matmul(
                        DP[:, n0 : n0 + nn], VV,
                        E[:, PLANE + h0 + n0 : PLANE + h0 + n0 + nn],
                        start=False, stop=True,
                    )
                nc.scalar.activation(L[:, h0 : h0 + HALF], DP[:], ACT.Ln)

            # softmax weight w_c = exp(p_c - ln(denom)), duplicated row blocks
            Z = pool.tile([NP, PLANE], bf16, tag="Z")
            nc.vector.tensor_tensor(Z[:], PR2[:], L[:], ALU.subtract)
            Wt = pool.tile([NP, PLANE], bf16, tag="Wt")
            nc.scalar.activation(Wt[:], Z[:], ACT.Exp)

            # ---- D pass: PE shift matmuls + DVE min accumulation ----
            FR = pool.tile([NP, PLANE], bf16, tag="FR")
            for j, s in enumerate(shifts):
                Wj = WM[0 : KROW + 1, j * 128 : j * 128 + 128]
                for h0 in range(0, PLANE, HALF):
                    SH = psp.tile([NP, HALF], f32, tag="ps")
                    for n0 in range(0, HALF, 512):
                        nn = min(512, HALF - n0)
                        nc.tensor.matmul(
                            SH[:, n0 : n0 + nn], Wj,
                            Cw[0 : KROW + 1, h0 + n0 : h0 + n0 + nn],
                            start=True, stop=True,
                        )
                    base = Cw if j == 0 else Dt
                    if j in RELAY:
                        nc.scalar.activation(
                            FR[0:RV, h0 : h0 + HALF], SH[0:RV, :], ACT.Copy
                        )
                        nc.vector.tensor_tensor(
                            Dt[0:RV, h0 : h0 + HALF], FR[0:RV, h0 : h0 + HALF],
                            base[0:RV, h0 : h0 + HALF], ALU.min,
                        )
                    else:
                        nc.vector.tensor_tensor(
                            Dt[0:RV, h0 : h0 + HALF], SH[0:RV, :],
                            base[0:RV, h0 : h0 + HALF], ALU.min,
                        )

            # ---- |sdf| = sqrt(g), weighted, out ----
            SQ = pool.tile([NP, PLANE], bf16, tag="SQ")
            nc.scalar.activation(SQ[0:RV, :], Dt[0:RV, :], ACT.Sqrt)
            T2 = pool.tile([RV, PLANE], bf16, tag="T2")
            nc.vector.tensor_tensor(T2[:], SQ[0:RV, :], Wt[0:RV, :], ALU.mult)
            nc.sync.dma_start(out_d[:], T2[:])

    nc.compile()
    return nc


def _pack_inputs(pred, tgt, S):
    """Host-side packing of per-core input tensors."""
    import ml_dtypes

    bf = ml_dtypes.bfloat16
    shifts = _shift_list(S)
    NSH = len(shifts)

    # shift matrices + denominator selector (shared by all cores)
    wm = np.zeros((NP, 128 * (NSH + 1)), np.float32)
    for j, s in enumerate(shifts):
        Ws = wm[:, j * 128 : (j + 1) * 128]
        for p in range(NP):
            if p < RV:
                blk = p // RB
                src = p - s
                lo = blk * RB
                if lo <= src < lo + RB:
                    Ws[src, p] = 1.0
                    Ws[KROW, p] = float(s * s)
                else:
                    Ws[KROW, p] = INF
            else:
                Ws[KROW, p] = INF
    VVm = wm[:, NSH * 128 : (NSH + 1) * 128]
    for p in range(NP):
        if p < RV:
            d = p % RB
            VVm[d, p] = 1.0
            VVm[RB + d, p] = 1.0
        else:
            VVm[p, p] = 1.0
    wm = wm.astype(bf)

    in_maps = []
    for k in range(N_CORES):
        b, c = divmod(k, C)
        t16 = tgt[b].reshape(DD, PLANE)
        T = np.zeros((NP, PLANE), np.int16)
        T[0:DD] = t16 != c        # pos seeds where t == c -> f=0 elsewhere INF
        T[RB : RB + DD] = t16 == c  # neg seeds where t != c

        others = [j for j in range(C) if j != c]
        pc = pred[b, c].reshape(DD, PLANE)
        o1, o2, o3 = (pred[b, j].reshape(DD, PLANE) for j in others)
        pr = np.zeros((NP, 2 * PLANE), np.float32)
        pr[0:DD, 0:PLANE] = pc
        pr[RB : RB + DD, 0:PLANE] = o1
        pr[0:DD, PLANE:] = o2
        pr[RB : RB + DD, PLANE:] = o3
        pr2 = np.zeros((NP, PLANE), np.float32)
        pr2[0:DD] = pc
        pr2[RB : RB + DD] = pc
        in_maps.append(
            {
                "tgt": T,
                "prden": pr.astype(bf),
                "prnum": pr2.astype(bf),
                "wmats": wm,
            }
        )
    return in_maps


def kernel(pred, target):
    pred = np.ascontiguousarray(np.asarray(pred), dtype=np.float32)
    target = np.asarray(target)

    if pred.shape != (B, C, DD, HH, WW) or target.shape != (B, DD, HH, WW):
        return _reference_fallback(pred, target)

    tgt = target.astype(np.int64)
    masks = []
    has_pos = {}
    for b in range(B):
        for c in range(C):
            m = tgt[b] == c
            has_pos[(b, c)] = bool(m.any())
            if has_pos[(b, c)]:
                masks.append(m)
                mn = ~m
                if mn.any():
                    masks.append(mn)
                else:
                    return _reference_fallback(pred, target)

    S = _certified_shift_bound(masks)
    if S > S_MAX:
        return _reference_fallback(pred, target)

    _ensure_paths()
    from concourse.bass_utils import run_bass_kernel_spmd

    if S not in _nc_cache:
        _nc_cache[S] = _build_nc(S)
    nc = _nc_cache[S]

    in_maps = _pack_inputs(pred, tgt, S)

    trace = bool(os.environ.get("BOUNDARY_KERNEL_TRACE"))
    if trace:
        import importlib.util

        if importlib.util.find_spec("antenv.axon_hooks") is None:
            trace = False
    res = run_bass_kernel_spmd(nc, in_maps, list(range(N_CORES)), trace=trace)
    global LAST_RESULTS
    LAST_RESULTS = res

    total = 0.0
    for k in range(N_CORES):
        b, c = divmod(k, C)
        if has_pos[(b, c)]:
            total += float(np.asarray(res.results[k]["t2"]).astype(np.float64).sum())
    return np.float32(total / (B * C * NVOX))


if __name__ == "__main__":
    import reference

    inputs = reference.setup_inputs()
    out = kernel(**{k: np.asarray(v) for k, v in inputs.items()})
    print("kernel out:", out)


# revision 6
# speedup vs baseline: 2.8233x; 1.0196x over previous
"""Trainium2 Bass kernel for nn_BoundaryLoss: mean(|softmax(pred) * SDF(onehot(target))|).

Strategy (8 NeuronCores, SPMD, one (b, c) pair per core):
  Exact 3D squared EDT of the class mask and its complement via separable
  truncated-shift min-plus passes (radius S certified exact on host against an
  untruncated reference EDT). Both volumes share one [128, 2304] bf16 tile,
  gapless: rows [0,48) pos (partition=d), rows [48,96) neg, free=(h,w); the
  host bakes mask polarity in, so the device sees a single 0/1 seed tensor.

  Engine plan (v2 — replaces the DMA-shift/DVE-heavy v1):
   - H/W passes on DVE as tensor_scalar (src+s^2, 4x mode) + tensor_tensor min
     (2x_1p); scalar_tensor_tensor only for the odd W-shifts that break 4B
     alignment anyway (STT has no fast uop, both run 1x).
   - D pass (partition axis) on the PE: per shift a [97x128] bf16 matrix whose
     columns pick row p-s and add s^2 (or 30000=INF) via a constant-1 row at
     partition 96; min-accumulated on DVE from PSUM (partly via a Scalar
     engine bf16 relay).
   - softmax weight = exp(p_c - ln(denom)); denom = PE matmul with a
     duplicating selector matrix (sums the 4 class planes into both row
     blocks); Ln/Exp/Sqrt on the Scalar engine. No DVE reciprocal.
   - per-voxel |sdf|*w_c tensor DMA'd out (bf16); final reduce on host.
"""

import os
import sys

import numpy as np

B, C, DD, HH, WW = 2, 4, 48, 48, 48
PLANE = HH * WW  # 2304
NVOX = DD * PLANE
NP = 128
RB = 48          # neg block start row (gapless: pos [0,48), neg [48,96))
RV = 96          # end of valid rows
KROW = 96        # constant-1 row for the shift matmuls (contract dim 97)
INF = 30000.0
S_MAX = 6        # fall back to numpy beyond this
N_CORES = 8
HALF = PLANE // 2  # 1152

_nc_cache = {}
LAST_RESULTS = None  # test harness introspection


def _ensure_paths():
    for p in ("/opt/trn_rl_repo",):
        if os.path.isdir(p) and p not in sys.path:
            sys.path.insert(0, p)


def _edt_sq_trunc_np(f0, S):
    """Truncated-shift separable squared EDT (numpy, int32)."""
    f = f0.astype(np.int32)
    for ax in (2, 1, 0):
        g = f.copy()
        for s in range(1, S + 1):
            s2 = s * s
            sl_out = [slice(None)] * 3
            sl_in = [slice(None)] * 3
            sl_out[ax] = slice(s, None)
            sl_in[ax] = slice(None, -s)
            np.minimum(g[tuple(sl_out)], f[tuple(sl_in)] + s2, out=g[tuple(sl_out)])
            sl_out[ax] = slice(None, -s)
            sl_in[ax] = slice(s, None)
            np.minimum(g[tuple(sl_out)], f[tuple(sl_in)] + s2, out=g[tuple(sl_out)])
        f = g
    return f


def _certified_shift_bound(masks):
    """Smallest S whose S-truncated separable EDT equals an exact reference on
    every mask. The reference uses S_ref from the conservative bound (max
    truncated distance <= S_ref implies exactness); smaller S are accepted iff
    they reproduce the reference exactly (a per-input certificate)."""
    refs = []
    S_ref = None
    for S in range(1, 17):
        worst = 0
        refs = []
        for m in masks:
            f0 = np.where(m, 0, 30000).astype(np.int16)
            g = _edt_sq_trunc_np(f0, S)
            refs.append(g)
            worst = max(worst, int(np.ceil(np.sqrt(float(g.max())))))
        if worst <= S:
            S_ref = S
            break
    if S_ref is None:
        return 17
    for S in range(1, S_ref):
        ok = True
        for m, ref in zip(masks, refs):
            f0 = np.where(m, 0, 30000).astype(np.int16)
            if not np.array_equal(_edt_sq_trunc_np(f0, S), ref):
                ok = False
                break
        if ok:
            return S
    return S_ref


def _reference_fallback(pred, target):
    """Exact numpy replica of the reference for pathological inputs."""
    BIG = 1e9
    pred = np.asarray(pred, np.float32)
    target = np.asarray(target)
    b_, c_ = pred.shape[0], pred.shape[1]
    n = np.arange(pred.shape[-1])

    def minplus(f):
        d2 = ((n[:, None] - n[None, :]) ** 2).astype(np.float32)
        return (f[..., None, :] + d2).min(axis=-1)

    def edt(src):
        f = np.where(src, 0.0, BIG).astype(np.float32)
        for ax in (-3, -2, -1):
            f = np.moveaxis(minplus(np.moveaxis(f, ax, -1)), -1, ax)
        return np.sqrt(f)

    e = np.exp(pred - pred.max(axis=1, keepdims=True))
    sm = e / e.sum(axis=1, keepdims=True)
    total = 0.0
    for b in range(b_):
        for c in range(c_):
            pos = target[b] == c
            if not pos.any():
                continue
            sdf = edt(pos) - edt(~pos)
            total += float(np.abs(sm[b, c] * sdf).sum(dtype=np.float64))
    return np.float32(total / pred.size)


def _shift_list(S):
    out = []
    for s in range(1, S + 1):
        out.append(s)
        out.append(-s)
    return out


def _build_nc(S):
    """Build + compile the SPMD Bass program for shift radius S."""
    _ensure_paths()
    import concourse.tile as tile
    from concourse import bacc, mybir

    i16 = mybir.dt.int16
    bf16 = mybir.dt.bfloat16
    f32 = mybir.dt.float32
    ALU = mybir.AluOpType
    ACT = mybir.ActivationFunctionType

    shifts = _shift_list(S)
    NSH = len(shifts)
    RELAY = set(range(min(3, NSH)))  # shift idxs relayed PSUM->SBUF on Scalar

    nc = bacc.Bacc("TRN2", target_bir_lowering=False, debug=False)

    sd_d = nc.dram_tensor("seed", [NP, PLANE], bf16, kind="ExternalInput")
    pr_d = nc.dram_tensor("prden", [NP, 2 * PLANE], bf16, kind="ExternalInput")
    pr2_d = nc.dram_tensor("prnum", [NP, PLANE], bf16, kind="ExternalInput")
    wm_d = nc.dram_tensor("wmats", [NP, 128 * (NSH + 1)], bf16, kind="ExternalInput")
    out_d = nc.dram_tensor("t2", [RV, PLANE], bf16, kind="ExternalOutput")

    HLVS = [(0, HH // 2), (HH // 2, HH)]  # h-ranges; free cols h*WW..(h+1)*WW

    with tile.TileContext(nc) as tc:
        with (
            tc.tile_pool(name="main", bufs=1) as pool,
            tc.tile_pool(name="psum", bufs=2, space="PSUM") as psp,
        ):
            A = pool.tile([NP, PLANE], bf16, tag="A")
            nc.sync.dma_start(A[:], sd_d[:])
            WM = pool.tile([NP, 128 * (NSH + 1)], bf16, tag="wm")
            nc.sync.dma_start(WM[:], wm_d[:])
            PR = pool.tile([NP, 2 * PLANE], bf16, tag="pr")
            nc.sync.dma_start(PR[:], pr_d[:])
            PR2 = pool.tile([NP, PLANE], bf16, tag="pr2")
            nc.sync.dma_start(PR2[:], pr2_d[:])

            Bh = pool.tile([NP, PLANE], bf16, tag="Bh")
            Cw = pool.tile([NP, PLANE], bf16, tag="Cw")
            Dt = pool.tile([NP, PLANE], bf16, tag="Dt")
            AS = {}
            for s in range(1, S + 1):
                AS[s] = pool.tile([NP, PLANE], bf16, tag=f"AS{s}", name=f"AS{s}")
            BS = {}
            for s in range(2, S + 1, 2):
                BS[s] = pool.tile([NP, PLANE], bf16, tag=f"BS{s}", name=f"BS{s}")

            a3 = A[:].rearrange("p (h w) -> p h w", w=WW)
            b3 = Bh[:].rearrange("p (h w) -> p h w", w=WW)
            c3 = Cw[:].rearrange("p (h w) -> p h w", w=WW)
            bh3 = Bh[:].rearrange("p (h w) -> p h w", w=WW)

            # ---- H pass (stride-48 shifts; first s=1 term replaces the copy)
            for s in range(1, S + 1):
                nc.vector.tensor_scalar(
                    out=AS[s][0:RV, :], in0=A[0:RV, :], scalar1=float(s * s),
                    scalar2=None, op0=ALU.add,
                )
            for h0, h1 in HLVS:
                as3 = AS[1][:].rearrange("p (h w) -> p h w", w=WW)
                lo = max(h0, 1)
                nc.vector.tensor_tensor(
                    b3[0:RV, lo:h1, :], as3[0:RV, lo - 1 : h1 - 1, :],
                    a3[0:RV, lo:h1, :], ALU.min,
                )
                if h0 == 0:  # h=0 edge: only the -1 neighbor exists
                    nc.vector.tensor_tensor(
                        b3[0:RV, 0:1, :], as3[0:RV, 1:2, :], a3[0:RV, 0:1, :],
                        ALU.min,
                    )
                nc.vector.tensor_tensor(
                    b3[0:RV, h0 : h1 - (1 if h1 == HH else 0), :],
                    as3[0:RV, h0 + 1 : h1 + (0 if h1 == HH else 1), :],
                    b3[0:RV, h0 : h1 - (1 if h1 == HH else 0), :], ALU.min,
                )
                for s in range(2, S + 1):
                    as3 = AS[s][:].rearrange("p (h w) -> p h w", w=WW)
                    lo = max(h0, s)
                    nc.vector.tensor_tensor(
                        b3[0:RV, lo:h1, :], as3[0:RV, lo - s : h1 - s, :],
                        b3[0:RV, lo:h1, :], ALU.min,
                    )
                    hi = min(h1, HH - s)
                    nc.vector.tensor_tensor(
                        b3[0:RV, h0:hi, :], as3[0:RV, h0 + s : hi + s, :],
                        b3[0:RV, h0:hi, :], ALU.min,
                    )

            # ---- W pass (innermost shifts; s=1 via STT, fused first term)
            for s in range(2, S + 1, 2):
                nc.vector.tensor_scalar(
                    out=BS[s][0:RV, :], in0=Bh[0:RV, :], scalar1=float(s * s),
                    scalar2=None, op0=ALU.add,
                )
            for h0, h1 in HLVS:
                nc.vector.scalar_tensor_tensor(
                    out=c3[0:RV, h0:h1, 1:], in0=bh3[0:RV, h0:h1, : WW - 1],
                    scalar=1.0, in1=bh3[0:RV, h0:h1, 1:], op0=ALU.add, op1=ALU.min,
                )
                nc.vector.scalar_tensor_tensor(
                    out=c3[0:RV, h0:h1, 0:1], in0=bh3[0:RV, h0:h1, 1:2],
                    scalar=1.0, in1=bh3[0:RV, h0:h1, 0:1], op0=ALU.add, op1=ALU.min,
                )
                nc.vector.scalar_tensor_tensor(
                    out=c3[0:RV, h0:h1, : WW - 1], in0=bh3[0:RV, h0:h1, 1:],
                    scalar=1.0, in1=c3[0:RV, h0:h1, : WW - 1],
                    op0=ALU.add, op1=ALU.min,
                )
                for s in range(2, S + 1):
                    if s % 2 == 0:
                        bs3 = BS[s][:].rearrange("p (h w) -> p h w", w=WW)
                        nc.vector.tensor_tensor(
                            c3[0:RV, h0:h1, s:], bs3[0:RV, h0:h1, : WW - s],
                            c3[0:RV, h0:h1, s:], ALU.min,
                        )
                        nc.vector.tensor_tensor(
                            c3[0:RV, h0:h1, : WW - s], bs3[0:RV, h0:h1, s:],
                            c3[0:RV, h0:h1, : WW - s], ALU.min,
                        )
                    else:
                        nc.vector.scalar_tensor_tensor(
                            out=c3[0:RV, h0:h1, s:], in0=bh3[0:RV, h0:h1, : WW - s],
                            scalar=float(s * s), in1=c3[0:RV, h0:h1, s:],
                            op0=ALU.add, op1=ALU.min,
                        )
                        nc.vector.scalar_tensor_tensor(
                            out=c3[0:RV, h0:h1, : WW - s], in0=bh3[0:RV, h0:h1, s:],
                            scalar=float(s * s), in1=c3[0:RV, h0:h1, : WW - s],
                            op0=ALU.add, op1=ALU.min,
                        )

            # constant-1 rows feeding the +s^2 / INF terms of the shift matmuls
            nc.gpsimd.memset(Cw[KROW:NP, :], 1.0)

            # ---- softmax denom on PE + Ln on Scalar (overlaps EDT) ----
            E = pool.tile([NP, 2 * PLANE], bf16, tag="E")
            nc.scalar.activation(E[:], PR[:], ACT.Exp)
            L = pool.tile([NP, PLANE], bf16, tag="L")
            VV = WM[:, NSH * 128 : (NSH + 1) * 128]
            for h0 in range(0, PLANE, HALF):
                DP = psp.tile([NP, HALF], f32, tag="ps")
                for n0 in range(0, HALF, 512):
                    nn = min(512, HALF - n0)
                    nc.tensor.matmul(
                        DP[:, n0 : n0 + nn], VV, E[:, h0 + n0 : h0 + n0 + nn],
                        start=True, stop=False,
                    )
                    nc.tensor.matmul(
                        DP[:, n0 : n0 + nn], VV,
                        E[:, PLANE + h0 + n0 : PLANE + h0 + n0 + nn],
                        start=False, stop=True,
                    )
                nc.scalar.activation(L[:, h0 : h0 + HALF], DP[:], ACT.Ln)

            # softmax weight w_c = exp(p_c - ln(denom)), duplicated row blocks
            Z = pool.tile([NP, PLANE], bf16, tag="Z")
            nc.vector.tensor_tensor(Z[:], PR2[:], L[:], ALU.subtract)
            Wt = pool.tile([NP, PLANE], bf16, tag="Wt")
            nc.scalar.activation(Wt[:], Z[:], ACT.Exp)

            # ---- D pass: PE shift matmuls + DVE min accumulation (per half)
            SQ = pool.tile([NP, PLANE], bf16, tag="SQ")
            T2 = pool.tile([RV, PLANE], bf16, tag="T2")
            FR = pool.tile([NP, PLANE], bf16, tag="FR")
            for hi, h0 in enumerate(range(0, PLANE, HALF)):
                for j, s in enumerate(shifts):
                    Wj = WM[0 : KROW + 1, j * 128 : j * 128 + 128]
                    SH = psp.tile([NP, HALF], f32, tag="ps")
                    for n0 in range(0, HALF, 512):
                        nn = min(512, HALF - n0)
                        nc.tensor.matmul(
                            SH[:, n0 : n0 + nn], Wj,
                            Cw[0 : KROW + 1, h0 + n0 : h0 + n0 + nn],
                            start=True, stop=True,
                        )
                    base = Cw if j == 0 else Dt
                    if j in RELAY:
                        nc.scalar.activation(
                            FR[0:RV, h0 : h0 + HALF], SH[0:RV, :], ACT.Copy
                        )
                        nc.vector.tensor_tensor(
                            Dt[0:RV, h0 : h0 + HALF], FR[0:RV, h0 : h0 + HALF],
                            base[0:RV, h0 : h0 + HALF], ALU.min,
                        )
                    else:
                        nc.vector.tensor_tensor(
                            Dt[0:RV, h0 : h0 + HALF], SH[0:RV, :],
                            base[0:RV, h0 : h0 + HALF], ALU.min,
                        )
                # ---- |sdf| = sqrt(g), weighted, out (pipelined per half) ----
                nc.scalar.activation(
                    SQ[0:RV, h0 : h0 + HALF], Dt[0:RV, h0 : h0 + HALF], ACT.Sqrt
                )
                nc.vector.tensor_tensor(
                    T2[:, h0 : h0 + HALF], SQ[0:RV, h0 : h0 + HALF],
                    Wt[0:RV, h0 : h0 + HALF], ALU.mult,
                )
                nc.sync.dma_start(
                    out_d[:, h0 : h0 + HALF], T2[:, h0 : h0 + HALF]
                )

    nc.compile()
    return nc


def _pack_inputs(pred, tgt, S):
    """Host-side packing of per-core input tensors."""
    import ml_dtypes

    bf = ml_dtypes.bfloat16
    shifts = _shift_list(S)
    NSH = len(shifts)

    # shift matrices + denominator selector (shared by all cores)
    wm = np.zeros((NP, 128 * (NSH + 1)), np.float32)
    for j, s in enumerate(shifts):
        Ws = wm[:, j * 128 : (j + 1) * 128]
        for p in range(NP):
            if p < RV:
                blk = p // RB
                src = p - s
                lo = blk * RB
                if lo <= src < lo + RB:
                    Ws[src, p] = 1.0
                    Ws[KROW, p] = float(s * s)
                else:
                    Ws[KROW, p] = INF
            else:
                Ws[KROW, p] = INF
    VVm = wm[:, NSH * 128 : (NSH + 1) * 128]
    for p in range(NP):
        if p < RV:
            d = p % RB
            VVm[d, p] = 1.0
            VVm[RB + d, p] = 1.0
        else:
            VVm[p, p] = 1.0
    wm = wm.astype(bf)

    in_maps = []
    for k in range(N_CORES):
        b, c = divmod(k, C)
        t16 = tgt[b].reshape(DD, PLANE)
        seed = np.zeros((NP, PLANE), np.float32)
        seed[0:DD] = (t16 != c) * INF     # pos: f=0 on class voxels
        seed[RB : RB + DD] = (t16 == c) * INF  # neg: f=0 off class voxels

        others = [j for j in range(C) if j != c]
        pc = pred[b, c].reshape(DD, PLANE)
        o1, o2, o3 = (pred[b, j].reshape(DD, PLANE) for j in others)
        pr = np.zeros((NP, 2 * PLANE), np.float32)
        pr[0:DD, 0:PLANE] = pc
        pr[RB : RB + DD, 0:PLANE] = o1
        pr[0:DD, PLANE:] = o2
        pr[RB : RB + DD, PLANE:] = o3
        pr2 = np.zeros((NP, PLANE), np.float32)
        pr2[0:DD] = pc
        pr2[RB : RB + DD] = pc
        in_maps.append(
            {
                "seed": seed.astype(bf),
                "prden": pr.astype(bf),
                "prnum": pr2.astype(bf),
                "wmats": wm,
            }
        )
    return in_maps


def kernel(pred, target):
    pred = np.ascontiguousarray(np.asarray(pred), dtype=np.float32)
    target = np.asarray(target)

    if pred.shape != (B, C, DD, HH, WW) or target.shape != (B, DD, HH, WW):
        return _reference_fallback(pred, target)

    tgt = target.astype(np.int64)
    masks = []
    has_pos = {}
    for b in range(B):
        for c in range(C):
            m = tgt[b] == c
            has_pos[(b, c)] = bool(m.any())
            if has_pos[(b, c)]:
                masks.append(m)
                mn = ~m
                if mn.any():
                    masks.append(mn)
                else:
                    return _reference_fallback(pred, target)

    S = _certified_shift_bound(masks)
    if S > S_MAX:
        return _reference_fallback(pred, target)

    _ensure_paths()
    from concourse.bass_utils import run_bass_kernel_spmd

    if S not in _nc_cache:
        _nc_cache[S] = _build_nc(S)
    nc = _nc_cache[S]

    in_maps = _pack_inputs(pred, tgt, S)

    trace = bool(os.environ.get("BOUNDARY_KERNEL_TRACE"))
    if trace:
        import importlib.util

        if importlib.util.find_spec("antenv.axon_hooks") is None:
            trace = False
    res = run_bass_kernel_spmd(nc, in_maps, list(range(N_CORES)), trace=trace)
    global LAST_RESULTS
    LAST_RESULTS = res

    total = 0.0
    for k in range(N_CORES):
        b, c = divmod(k, C)
        if has_pos[(b, c)]:
            total += float(np.asarray(res.results[k]["t2"]).astype(np.float64).sum())
    return np.float32(total / (B * C * NVOX))


if __name__ == "__main__":
    import reference

    inputs = reference.setup_inputs()
    out = kernel(**{k: np.asarray(v) for k, v in inputs.items()})
    print("kernel out:", out)


# revision 7
# speedup vs baseline: 2.8246x; 1.0005x over previous
"""Trainium2 Bass kernel for nn_BoundaryLoss: mean(|softmax(pred) * SDF(onehot(target))|).

Strategy (8 NeuronCores, SPMD, one (b, c) pair per core):
  Exact 3D squared EDT of the class mask and its complement via separable
  truncated-shift min-plus passes (radius S certified exact on host against an
  untruncated reference EDT). Both volumes share one [128, 2304] bf16 tile,
  gapless: rows [0,48) pos (partition=d), rows [48,96) neg, free=(h,w); the
  host bakes mask polarity in, so the device sees a single 0/1 seed tensor.

  Engine plan (v2 — replaces the DMA-shift/DVE-heavy v1):
   - H/W passes on DVE as tensor_scalar (src+s^2, 4x mode) + tensor_tensor min
     (2x_1p); scalar_tensor_tensor only for the odd W-shifts that break 4B
     alignment anyway (STT has no fast uop, both run 1x).
   - D pass (partition axis) on the PE: per shift a [97x128] bf16 matrix whose
     columns pick row p-s and add s^2 (or 30000=INF) via a constant-1 row at
     partition 96; min-accumulated on DVE from PSUM (partly via a Scalar
     engine bf16 relay).
   - softmax weight = exp(p_c - ln(denom)); denom = PE matmul with a
     duplicating selector matrix (sums the 4 class planes into both row
     blocks); Ln/Exp/Sqrt on the Scalar engine. No DVE reciprocal.
   - per-voxel |sdf|*w_c tensor DMA'd out (bf16); final reduce on host.
"""

import os
import sys

import numpy as np

B, C, DD, HH, WW = 2, 4, 48, 48, 48
PLANE = HH * WW  # 2304
NVOX = DD * PLANE
NP = 128
RB = 48          # neg block start row (gapless: pos [0,48), neg [48,96))
RV = 96          # end of valid rows
KROW = 96        # constant-1 row for the shift matmuls (contract dim 97)
INF = 30000.0
S_MAX = 6        # fall back to numpy beyond this
N_CORES = 8
HALF = PLANE // 2  # 1152

_nc_cache = {}
LAST_RESULTS = None  # test harness introspection


def _ensure_paths():
    for p in ("/opt/trn_rl_repo",):
        if os.path.isdir(p) and p not in sys.path:
            sys.path.insert(0, p)


def _edt_sq_trunc_np(f0, S):
    """Truncated-shift separable squared EDT (numpy, int32)."""
    f = f0.astype(np.int32)
    for ax in (2, 1, 0):
        g = f.copy()
        for s in range(1, S + 1):
            s2 = s * s
            sl_out = [slice(None)] * 3
            sl_in = [slice(None)] * 3
            sl_out[ax] = slice(s, None)
            sl_in[ax] = slice(None, -s)
            np.minimum(g[tuple(sl_out)], f[tuple(sl_in)] + s2, out=g[tuple(sl_out)])
            sl_out[ax] = slice(None, -s)
            sl_in[ax] = slice(s, None)
            np.minimum(g[tuple(sl_out)], f[tuple(sl_in)] + s2, out=g[tuple(sl_out)])
        f = g
    return f


def _certified_shift_bound(masks):
    """Smallest S whose S-truncated separable EDT equals an exact reference on
    every mask. The reference uses S_ref from the conservative bound (max
    truncated distance <= S_ref implies exactness); smaller S are accepted iff
    they reproduce the reference exactly (a per-input certificate)."""
    refs = []
    S_ref = None
    for S in range(1, 17):
        worst = 0
        refs = []
        for m in masks:
            f0 = np.where(m, 0, 30000).astype(np.int16)
            g = _edt_sq_trunc_np(f0, S)
            refs.append(g)
            worst = max(worst, int(np.ceil(np.sqrt(float(g.max())))))
        if worst <= S:
            S_ref = S
            break
    if S_ref is None:
        return 17
    for S in range(1, S_ref):
        ok = True
        for m, ref in zip(masks, refs):
            f0 = np.where(m, 0, 30000).astype(np.int16)
            if not np.array_equal(_edt_sq_trunc_np(f0, S), ref):
                ok = False
                break
        if ok:
            return S
    return S_ref


def _reference_fallback(pred, target):
    """Exact numpy replica of the reference for pathological inputs."""
    BIG = 1e9
    pred = np.asarray(pred, np.float32)
    target = np.asarray(target)
    b_, c_ = pred.shape[0], pred.shape[1]
    n = np.arange(pred.shape[-1])

    def minplus(f):
        d2 = ((n[:, None] - n[None, :]) ** 2).astype(np.float32)
        return (f[..., None, :] + d2).min(axis=-1)

    def edt(src):
        f = np.where(src, 0.0, BIG).astype(np.float32)
        for ax in (-3, -2, -1):
            f = np.moveaxis(minplus(np.moveaxis(f, ax, -1)), -1, ax)
        return np.sqrt(f)

    e = np.exp(pred - pred.max(axis=1, keepdims=True))
    sm = e / e.sum(axis=1, keepdims=True)
    total = 0.0
    for b in range(b_):
        for c in range(c_):
            pos = target[b] == c
            if not pos.any():
                continue
            sdf = edt(pos) - edt(~pos)
            total += float(np.abs(sm[b, c] * sdf).sum(dtype=np.float64))
    return np.float32(total / pred.size)


def _shift_list(S):
    out = []
    for s in range(1, S + 1):
        out.append(s)
        out.append(-s)
    return out


def _build_nc(S):
    """Build + compile the SPMD Bass program for shift radius S."""
    _ensure_paths()
    import concourse.tile as tile
    from concourse import bacc, mybir

    i16 = mybir.dt.int16
    bf16 = mybir.dt.bfloat16
    f32 = mybir.dt.float32
    ALU = mybir.AluOpType
    ACT = mybir.ActivationFunctionType

    shifts = _shift_list(S)
    NSH = len(shifts)
    RELAY = set(range(min(3, NSH)))  # shift idxs relayed PSUM->SBUF on Scalar

    nc = bacc.Bacc("TRN2", target_bir_lowering=False, debug=False)

    sd_d = nc.dram_tensor("seed", [NP, PLANE], bf16, kind="ExternalInput")
    pr_d = nc.dram_tensor("prden", [NP, 2 * PLANE], bf16, kind="ExternalInput")
    pr2_d = nc.dram_tensor("prnum", [NP, PLANE], bf16, kind="ExternalInput")
    wm_d = nc.dram_tensor("wmats", [NP, 128 * (NSH + 1)], bf16, kind="ExternalInput")
    out_d = nc.dram_tensor("t2", [RV, PLANE], bf16, kind="ExternalOutput")

    HLVS = [(0, HH // 2), (HH // 2, HH)]  # h-ranges; free cols h*WW..(h+1)*WW

    with tile.TileContext(nc) as tc:
        with (
            tc.tile_pool(name="main", bufs=1) as pool,
            tc.tile_pool(name="psum", bufs=2, space="PSUM") as psp,
        ):
            A = pool.tile([NP, PLANE], bf16, tag="A")
            nc.sync.dma_start(A[:], sd_d[:])
            WM = pool.tile([NP, 128 * (NSH + 1)], bf16, tag="wm")
            nc.sync.dma_start(WM[:], wm_d[:])
            PR = pool.tile([NP, 2 * PLANE], bf16, tag="pr")
            nc.sync.dma_start(PR[:], pr_d[:])
            PR2 = pool.tile([NP, PLANE], bf16, tag="pr2")
            nc.sync.dma_start(PR2[:], pr2_d[:])

            Bh = pool.tile([NP, PLANE], bf16, tag="Bh")
            Cw = pool.tile([NP, PLANE], bf16, tag="Cw")
            Dt = pool.tile([NP, PLANE], bf16, tag="Dt")
            AS = {}
            for s in range(1, S + 1):
                AS[s] = pool.tile([NP, PLANE], bf16, tag=f"AS{s}", name=f"AS{s}")
            BS = {}
            for s in range(2, S + 1, 2):
                BS[s] = pool.tile([NP, PLANE], bf16, tag=f"BS{s}", name=f"BS{s}")

            # ---- softmax denom on PE + Ln on Scalar (overlaps EDT) ----
            E = pool.tile([NP, 2 * PLANE], bf16, tag="E")
            nc.scalar.activation(E[:], PR[:], ACT.Exp)
            L = pool.tile([NP, PLANE], bf16, tag="L")
            VV = WM[:, NSH * 128 : (NSH + 1) * 128]
            for h0 in range(0, PLANE, HALF):
                DP = psp.tile([NP, HALF], f32, tag="ps")
                for n0 in range(0, HALF, 512):
                    nn = min(512, HALF - n0)
                    nc.tensor.matmul(
                        DP[:, n0 : n0 + nn], VV, E[:, h0 + n0 : h0 + n0 + nn],
                        start=True, stop=False,
                    )
                    nc.tensor.matmul(
                        DP[:, n0 : n0 + nn], VV,
                        E[:, PLANE + h0 + n0 : PLANE + h0 + n0 + nn],
                        start=False, stop=True,
                    )
                nc.scalar.activation(L[:, h0 : h0 + HALF], DP[:], ACT.Ln)

            # softmax weight w_c = exp(p_c - ln(denom)), duplicated row blocks
            Z = pool.tile([NP, PLANE], bf16, tag="Z")
            nc.vector.tensor_tensor(Z[:], PR2[:], L[:], ALU.subtract)
            Wt = pool.tile([NP, PLANE], bf16, tag="Wt")
            nc.scalar.activation(Wt[:], Z[:], ACT.Exp)

            a3 = A[:].rearrange("p (h w) -> p h w", w=WW)
            b3 = Bh[:].rearrange("p (h w) -> p h w", w=WW)
            c3 = Cw[:].rearrange("p (h w) -> p h w", w=WW)
            bh3 = Bh[:].rearrange("p (h w) -> p h w", w=WW)

            # ---- H pass (stride-48 shifts; first s=1 term replaces the copy)
            for s in range(1, S + 1):
                nc.vector.tensor_scalar(
                    out=AS[s][0:RV, :], in0=A[0:RV, :], scalar1=float(s * s),
                    scalar2=None, op0=ALU.add,
                )
            for h0, h1 in HLVS:
                as3 = AS[1][:].rearrange("p (h w) -> p h w", w=WW)
                lo = max(h0, 1)
                nc.vector.tensor_tensor(
                    b3[0:RV, lo:h1, :], as3[0:RV, lo - 1 : h1 - 1, :],
                    a3[0:RV, lo:h1, :], ALU.min,
                )
                if h0 == 0:  # h=0 edge: only the -1 neighbor exists
                    nc.vector.tensor_tensor(
                        b3[0:RV, 0:1, :], as3[0:RV, 1:2, :], a3[0:RV, 0:1, :],
                        ALU.min,
                    )
                nc.vector.tensor_tensor(
                    b3[0:RV, h0 : h1 - (1 if h1 == HH else 0), :],
                    as3[0:RV, h0 + 1 : h1 + (0 if h1 == HH else 1), :],
                    b3[0:RV, h0 : h1 - (1 if h1 == HH else 0), :], ALU.min,
                )
                for s in range(2, S + 1):
                    as3 = AS[s][:].rearrange("p (h w) -> p h w", w=WW)
                    lo = max(h0, s)
                    nc.vector.tensor_tensor(
                        b3[0:RV, lo:h1, :], as3[0:RV, lo - s : h1 - s, :],
                        b3[0:RV, lo:h1, :], ALU.min,
                    )
                    hi = min(h1, HH - s)
                    nc.vector.tensor_tensor(
                        b3[0:RV, h0:hi, :], as3[0:RV, h0 + s : hi + s, :],
                        b3[0:RV, h0:hi, :], ALU.min,
                    )

            # ---- W pass (innermost shifts; s=1 via STT, fused first term)
            for s in range(2, S + 1, 2):
                nc.vector.tensor_scalar(
                    out=BS[s][0:RV, :], in0=Bh[0:RV, :], scalar1=float(s * s),
                    scalar2=None, op0=ALU.add,
                )
            for h0, h1 in HLVS:
                nc.vector.scalar_tensor_tensor(
                    out=c3[0:RV, h0:h1, 1:], in0=bh3[0:RV, h0:h1, : WW - 1],
                    scalar=1.0, in1=bh3[0:RV, h0:h1, 1:], op0=ALU.add, op1=ALU.min,
                )
                nc.vector.scalar_tensor_tensor(
                    out=c3[0:RV, h0:h1, 0:1], in0=bh3[0:RV, h0:h1, 1:2],
                    scalar=1.0, in1=bh3[0:RV, h0:h1, 0:1], op0=ALU.add, op1=ALU.min,
                )
                nc.vector.scalar_tensor_tensor(
                    out=c3[0:RV, h0:h1, : WW - 1], in0=bh3[0:RV, h0:h1, 1:],
                    scalar=1.0, in1=c3[0:RV, h0:h1, : WW - 1],
                    op0=ALU.add, op1=ALU.min,
                )
                for s in range(2, S + 1):
                    if s % 2 == 0:
                        bs3 = BS[s][:].rearrange("p (h w) -> p h w", w=WW)
                        nc.vector.tensor_tensor(
                            c3[0:RV, h0:h1, s:], bs3[0:RV, h0:h1, : WW - s],
                            c3[0:RV, h0:h1, s:], ALU.min,
                        )
                        nc.vector.tensor_tensor(
                            c3[0:RV, h0:h1, : WW - s], bs3[0:RV, h0:h1, s:],
                            c3[0:RV, h0:h1, : WW - s], ALU.min,
                        )
                    else:
                        nc.vector.scalar_tensor_tensor(
                            out=c3[0:RV, h0:h1, s:], in0=bh3[0:RV, h0:h1, : WW - s],
                            scalar=float(s * s), in1=c3[0:RV, h0:h1, s:],
                            op0=ALU.add, op1=ALU.min,
                        )
                        nc.vector.scalar_tensor_tensor(
                            out=c3[0:RV, h0:h1, : WW - s], in0=bh3[0:RV, h0:h1, s:],
                            scalar=float(s * s), in1=c3[0:RV, h0:h1, : WW - s],
                            op0=ALU.add, op1=ALU.min,
                        )

            # constant-1 rows feeding the +s^2 / INF terms of the shift matmuls
            nc.gpsimd.memset(Cw[KROW:NP, :], 1.0)

            # ---- D pass: PE shift matmuls + DVE min accumulation (per half)
            SQ = pool.tile([NP, PLANE], bf16, tag="SQ")
            T2 = pool.tile([RV, PLANE], bf16, tag="T2")
            FR = pool.tile([NP, PLANE], bf16, tag="FR")
            for hi, h0 in enumerate(range(0, PLANE, HALF)):
                for j, s in enumerate(shifts):
                    Wj = WM[0 : KROW + 1, j * 128 : j * 128 + 128]
                    SH = psp.tile([NP, HALF], f32, tag="ps")
                    for n0 in range(0, HALF, 512):
                        nn = min(512, HALF - n0)
                        nc.tensor.matmul(
                            SH[:, n0 : n0 + nn], Wj,
                            Cw[0 : KROW + 1, h0 + n0 : h0 + n0 + nn],
                            start=True, stop=True,
                        )
                    base = Cw if j == 0 else Dt
                    if j in RELAY:
                        nc.scalar.activation(
                            FR[0:RV, h0 : h0 + HALF], SH[0:RV, :], ACT.Copy
                        )
                        nc.vector.tensor_tensor(
                            Dt[0:RV, h0 : h0 + HALF], FR[0:RV, h0 : h0 + HALF],
                            base[0:RV, h0 : h0 + HALF], ALU.min,
                        )
                    else:
                        nc.vector.tensor_tensor(
                            Dt[0:RV, h0 : h0 + HALF], SH[0:RV, :],
                            base[0:RV, h0 : h0 + HALF], ALU.min,
                        )
                # ---- |sdf| = sqrt(g), weighted, out (pipelined per half) ----
                nc.scalar.activation(
                    SQ[0:RV, h0 : h0 + HALF], Dt[0:RV, h0 : h0 + HALF], ACT.Sqrt
                )
                nc.vector.tensor_tensor(
                    T2[:, h0 : h0 + HALF], SQ[0:RV, h0 : h0 + HALF],
                    Wt[0:RV, h0 : h0 + HALF], ALU.mult,
                )
                nc.sync.dma_start(
                    out_d[:, h0 : h0 + HALF], T2[:, h0 : h0 + HALF]
                )

    nc.compile()
    return nc


def _pack_inputs(pred, tgt, S):
    """Host-side packing of per-core input tensors."""
    import ml_dtypes

    bf = ml_dtypes.bfloat16
    shifts = _shift_list(S)
    NSH = len(shifts)

    # shift matrices + denominator selector (shared by all cores)
    wm = np.zeros((NP, 128 * (NSH + 1)), np.float32)
    for j, s in enumerate(shifts):
        Ws = wm[:, j * 128 : (j + 1) * 128]
        for p in range(NP):
            if p < RV:
                blk = p // RB
                src = p - s
                lo = blk * RB
                if lo <= src < lo + RB:
                    Ws[src, p] = 1.0
                    Ws[KROW, p] = float(s * s)
                else:
                    Ws[KROW, p] = INF
            else:
                Ws[KROW, p] = INF
    VVm = wm[:, NSH * 128 : (NSH + 1) * 128]
    for p in range(NP):
        if p < RV:
            d = p % RB
            VVm[d, p] = 1.0
            VVm[RB + d, p] = 1.0
        else:
            VVm[p, p] = 1.0
    wm = wm.astype(bf)

    in_maps = []
    for k in range(N_CORES):
        b, c = divmod(k, C)
        t16 = tgt[b].reshape(DD, PLANE)
        seed = np.zeros((NP, PLANE), np.float32)
        seed[0:DD] = (t16 != c) * INF     # pos: f=0 on class voxels
        seed[RB : RB + DD] = (t16 == c) * INF  # neg: f=0 off class voxels

        others = [j for j in range(C) if j != c]
        pc = pred[b, c].reshape(DD, PLANE)
        o1, o2, o3 = (pred[b, j].reshape(DD, PLANE) for j in others)
        pr = np.zeros((NP, 2 * PLANE), np.float32)
        pr[0:DD, 0:PLANE] = pc
        pr[RB : RB + DD, 0:PLANE] = o1
        pr[0:DD, PLANE:] = o2
        pr[RB : RB + DD, PLANE:] = o3
        pr2 = np.zeros((NP, PLANE), np.float32)
        pr2[0:DD] = pc
        pr2[RB : RB + DD] = pc
        in_maps.append(
            {
                "seed": seed.astype(bf),
                "prden": pr.astype(bf),
                "prnum": pr2.astype(bf),
                "wmats": wm,
            }
        )
    return in_maps


def kernel(pred, target):
    pred = np.ascontiguousarray(np.asarray(pred), dtype=np.float32)
    target = np.asarray(target)

    if pred.shape != (B, C, DD, HH, WW) or target.shape != (B, DD, HH, WW):
        return _reference_fallback(pred, target)

    tgt = target.astype(np.int64)
    masks = []
    has_pos = {}
    for b in range(B):
        for c in range(C):
            m = tgt[b] == c
            has_pos[(b, c)] = bool(m.any())
            if has_pos[(b, c)]:
                masks.append(m)
                mn = ~m
                if mn.any():
                    masks.append(mn)
                else:
                    return _reference_fallback(pred, target)

    S = _certified_shift_bound(masks)
    if S > S_MAX:
        return _reference_fallback(pred, target)

    _ensure_paths()
    from concourse.bass_utils import run_bass_kernel_spmd

    if S not in _nc_cache:
        _nc_cache[S] = _build_nc(S)
    nc = _nc_cache[S]

    in_maps = _pack_inputs(pred, tgt, S)

    trace = bool(os.environ.get("BOUNDARY_KERNEL_TRACE"))
    if trace:
        import importlib.util

        if importlib.util.find_spec("antenv.axon_hooks") is None:
            trace = False
    res = run_bass_kernel_spmd(nc, in_maps, list(range(N_CORES)), trace=trace)
    global LAST_RESULTS
    LAST_RESULTS = res

    total = 0.0
    for k in range(N_CORES):
        b, c = divmod(k, C)
        if has_pos[(b, c)]:
            total += float(np.asarray(res.results[k]["t2"]).astype(np.float64).sum())
    return np.float32(total / (B * C * NVOX))


if __name__ == "__main__":
    import reference

    inputs = reference.setup_inputs()
    out = kernel(**{k: np.asarray(v) for k, v in inputs.items()})
    print("kernel out:", out)
